# revision 2
# baseline (speedup 1.0000x reference)
"""BiMamba (fwd+bwd Mamba2 + fusion Linear) Trainium2 kernel — v2.

Sharding: 8 cores = 2 branches x 4 batches. Each core runs one full Mamba2
branch on one batch element via the chunked SSD formulation (chunk=128), with
the out-proj and fusion Linear folded into one matmul (W_comb). Host flips x
for the backward branch and sums the two per-branch partial outputs.

v2 changes vs v1:
- state update batched: 3 matmuls of N=512 instead of 24 per-head N=64
- mask exp bias folded into the broadcast matmul (48 bias rows hi/lo via a
  one-hot rhs block), one Exp per 8 heads instead of one per head
- conv: single batched SiLU per L-tile, batched halo handling, alignment-aware
  tap order
- batched PSUM evacuation of xs/yn transposes (4 transposes per copy)
- ywork in bf16; misc op batching
"""

import sys

sys.path.insert(0, "/opt/trn_rl_repo")

import numpy as np
import ml_dtypes

D_MODEL = 768
D_STATE = 64
D_CONV = 4
D_INNER = 1536
HEADDIM = 64
H = 24
CONV_DIM = D_INNER + 2 * D_STATE  # 1664
D_IN_PROJ = 2 * D_INNER + 2 * D_STATE + H  # 3224
D_IN_EXT = D_IN_PROJ + 40  # 3264: dt cols [3200:3264] = dt,pad8,dt,pad8
BATCH, SEQ = 4, 4096

LT = 512  # L-tile
NLT = SEQ // LT  # 8
Q = 128  # chunk
NCH = LT // Q  # chunks per L-tile
KT = D_MODEL // 128  # 6 k-tiles
MX = 13  # xBC m-tiles (1664/128)
NZ = D_INNER // 512  # 3 z slices
XROW = LT + 4  # padded xin row stride (keeps taps 4B-aligned)

_CACHE = {}


def _build_nc():
    import concourse.bass as bass
    import concourse.bacc as bacc
    import concourse.mybir as mybir
    from concourse.tile import TileContext
    from concourse.masks import make_identity

    fp32 = mybir.dt.float32
    bf16 = mybir.dt.bfloat16
    AX = mybir.AluOpType
    AF = mybir.ActivationFunctionType

    nc = bacc.Bacc("TRN2", debug=False, num_devices=8)

    xT = nc.declare_dram_parameter("xT", [D_MODEL, SEQ], bf16, isOutput=False)
    wip = nc.declare_dram_parameter("wip", [D_MODEL, D_IN_EXT], bf16, isOutput=False)
    wcb = nc.declare_dram_parameter("wcb", [D_INNER, D_MODEL], bf16, isOutput=False)
    cw = nc.declare_dram_parameter("cw", [CONV_DIM, D_CONV], fp32, isOutput=False)
    cb = nc.declare_dram_parameter("cb", [CONV_DIM], fp32, isOutput=False)
    dtb = nc.declare_dram_parameter("dtb", [64, 1], fp32, isOutput=False)
    apos = nc.declare_dram_parameter("apos", [64, 1], fp32, isOutput=False)
    dsb = nc.declare_dram_parameter("dsb", [128, D_INNER], bf16, isOutput=False)
    oh = nc.declare_dram_parameter("oh", [64, H * Q], bf16, isOutput=False)
    out = nc.declare_dram_parameter("out", [SEQ, D_MODEL], fp32, isOutput=True)

    with TileContext(nc) as tc:
        with (
            tc.tile_pool(name="const", bufs=1) as cpool,
            tc.tile_pool(name="xt", bufs=2) as xtpool,
            tc.tile_pool(name="work", bufs=2) as wpool,
            tc.tile_pool(name="conv", bufs=2) as convpool,
            tc.tile_pool(name="convin", bufs=1) as cinpool,
            tc.tile_pool(name="ssd", bufs=2) as spool,
            tc.tile_pool(name="ssdsm", bufs=4) as smpool,
            tc.tile_pool(name="state", bufs=2) as statepool,
            tc.tile_pool(name="dram", bufs=2, space="DRAM") as dpool,
            tc.tile_pool(name="pbig", bufs=3, space="PSUM") as pbig,
            tc.tile_pool(name="pmask", bufs=1, space="PSUM") as pmaskp,
            tc.tile_pool(name="ptr", bufs=2, space="PSUM") as ptrp,
            tc.tile_pool(name="psm", bufs=1, space="PSUM") as psm,
        ):
            # ---- constants ----
            wip_sb = cpool.tile([128, KT, D_IN_EXT], bf16, tag="wip")
            nc.sync.dma_start(
                out=wip_sb[:], in_=wip.ap().rearrange("(k p) m -> p k m", p=128)
            )
            wcb_sb = cpool.tile([128, 12, D_MODEL], bf16, tag="wcb")
            nc.sync.dma_start(
                out=wcb_sb[:], in_=wcb.ap().rearrange("(j p) m -> p j m", p=128)
            )
            cw_sb = cpool.tile([128, MX, D_CONV], fp32, tag="cw")
            nc.sync.dma_start(
                out=cw_sb[:], in_=cw.ap().rearrange("(a p) k -> p a k", p=128)
            )
            cb_sb = cpool.tile([128, MX], fp32, tag="cb")
            nc.sync.dma_start(
                out=cb_sb[:], in_=cb.ap().rearrange("(a p) -> p a", p=128)
            )
            dtb_sb = cpool.tile([64, 1], fp32, tag="dtb")
            nc.sync.dma_start(out=dtb_sb[:], in_=dtb.ap())
            apos_sb = cpool.tile([64, 1], fp32, tag="apos")
            nc.sync.dma_start(out=apos_sb[:], in_=apos.ap())
            dsb_sb = cpool.tile([128, D_INNER], bf16, tag="dsb")
            nc.sync.dma_start(out=dsb_sb[:], in_=dsb.ap())

            # persistent mask-matmul rhs tiles (double-buffered by chunk parity):
            # rows 0-47 one-hot head selector (constant), rows 48-49 hilo (per chunk)
            rhs50 = cpool.tile([66, H * Q], bf16, tag="rhs50")
            nc.sync.dma_start(out=rhs50[0:64, :], in_=oh.ap())

            ident_b = cpool.tile([128, 128], bf16, tag="idb")
            make_identity(nc, ident_b[:])
            ident_f = cpool.tile([32, 32], fp32, tag="idf")
            make_identity(nc, ident_f[:])
            pones2 = cpool.tile([2, 64], bf16, tag="pones2")
            nc.gpsimd.memset(pones2[:], 1.0)
            zer48 = cpool.tile([64, 128], bf16, tag="zer48")
            nc.gpsimd.memset(zer48[:], 0.0)
            eps_c = cpool.tile([128, 1], fp32, tag="eps")
            nc.gpsimd.memset(eps_c[:], 1e-5)
            one48 = cpool.tile([64, 1], fp32, tag="one48")
            nc.gpsimd.memset(one48[:], 1.0)

            # ---- loop-carried state ----
            S_f = statepool.tile([64, H * 64], fp32, tag="Sf")
            nc.vector.memset(S_f[:], 0.0)
            S_b = statepool.tile([64, H * 64], bf16, tag="Sb")
            nc.vector.memset(S_b[:], 0.0)

            halo = convpool.tile([128, MX, 3], bf16, tag="halo", name="halo")
            nc.vector.memset(halo[:], 0.0)

            for li in range(NLT):
                l0 = li * LT
                xtt = xtpool.tile([128, KT, LT], bf16, tag="xtt")
                nc.sync.dma_start(
                    out=xtt[:],
                    in_=xT.ap().rearrange("(k p) l -> p k l", p=128)[:, :, l0 : l0 + LT],
                )

                # ---- in_proj: xBC part (channel layout) ----
                xin = cinpool.tile([128, MX, XROW], bf16, tag="xin")
                nc.any.tensor_copy(xin[:, :, 0:3], halo[:])
                for m in range(MX):
                    ps = pbig.tile([128, LT], fp32, tag="big")
                    for k in range(KT):
                        nc.tensor.matmul(
                            ps[:],
                            lhsT=wip_sb[:, k, D_INNER + m * 128 : D_INNER + (m + 1) * 128],
                            rhs=xtt[:, k, :],
                            start=(k == 0),
                            stop=(k == KT - 1),
                        )
                    nc.any.tensor_copy(xin[:, m, 3 : LT + 3], ps[:])
                # new halo for next L-tile
                halo_new = convpool.tile([128, MX, 3], bf16, tag="halo")
                nc.any.tensor_copy(halo_new[:], xin[:, :, LT : LT + 3])
                halo = halo_new

                # ---- conv: 4 taps, tap3 via tensor_scalar (+bias), taps 2/1/0 STT
                co_all = convpool.tile([128, MX, LT], bf16, tag="co")
                for m in range(MX):
                    nc.vector.tensor_scalar(
                        out=co_all[:, m, :],
                        in0=xin[:, m, 3 : LT + 3],
                        scalar1=cw_sb[:, m, 3:4],
                        scalar2=cb_sb[:, m : m + 1],
                        op0=AX.mult,
                        op1=AX.add,
                    )
                    for k in (2, 0, 1):
                        nc.vector.scalar_tensor_tensor(
                            out=co_all[:, m, :],
                            in0=xin[:, m, k : k + LT],
                            scalar=cw_sb[:, m, k : k + 1],
                            in1=co_all[:, m, :],
                            op0=AX.mult,
                            op1=AX.add,
                        )
                nc.scalar.activation(co_all[:], co_all[:], AF.Silu)

                Bt = convpool.tile([64, LT], bf16, tag="Bt")
                nc.any.tensor_copy(Bt[:], co_all[0:64, 12, :])
                Ct = convpool.tile([64, LT], bf16, tag="Ct")
                nc.sync.dma_start(out=Ct[:], in_=co_all[64:128, 12, :])

                # ---- z part in_proj (token layout) + silu, whole L-tile ----
                sz_lt = wpool.tile([128, NCH, D_INNER], bf16, tag="szlt", bufs=1)
                for c4 in range(NCH):
                    c4s = slice(c4 * Q, (c4 + 1) * Q)
                    for j in range(NZ):
                        pz = pbig.tile([128, 512], fp32, tag="big")
                        for k in range(KT):
                            nc.tensor.matmul(
                                pz[:],
                                lhsT=xtt[:, k, c4s],
                                rhs=wip_sb[:, k, j * 512 : (j + 1) * 512],
                                start=(k == 0), stop=(k == KT - 1),
                            )
                        nc.scalar.activation(
                            sz_lt[:, c4, j * 512 : (j + 1) * 512], pz[:], AF.Silu
                        )

                # ---- in_proj: dt part (48 duplicated rows, channel layout) ----
                psdt = pbig.tile([64, LT], fp32, tag="big")
                for k in range(KT):
                    nc.tensor.matmul(
                        psdt[:],
                        lhsT=wip_sb[:, k, D_INNER + CONV_DIM : D_IN_EXT],
                        rhs=xtt[:, k, :],
                        start=(k == 0),
                        stop=(k == KT - 1),
                    )
                art = wpool.tile([64, LT], fp32, tag="art", bufs=1)
                nc.scalar.activation(art[:], psdt[:], AF.Exp, bias=dtb_sb[:])
                dtt = wpool.tile([64, LT], fp32, tag="dtt", bufs=1)
                nc.scalar.activation(dtt[:], art[:], AF.Ln, bias=one48[:])
                nc.vector.tensor_scalar(
                    out=art[:], in0=dtt[:], scalar1=apos_sb[:],
                    scalar2=None, op0=AX.mult,
                )
                cnt = wpool.tile([64, LT], fp32, tag="cnt", bufs=1)
                for c in range(NCH):
                    cs = slice(c * Q, (c + 1) * Q)
                    nc.vector.tensor_tensor_scan(
                        out=cnt[:, cs],
                        data0=art[:, cs],
                        data1=zer48[:],
                        initial=0.0,
                        op0=AX.add,
                        op1=AX.add,
                    )

                # biasc(rows 0-47) = ln(dt) + cnt (hi in 0-23, lo in 24-47 later)
                biasc = wpool.tile([64, LT], fp32, tag="art", bufs=1)
                nc.scalar.activation(biasc[:], dtt[:], AF.Ln)
                nc.vector.tensor_tensor(
                    out=biasc[:], in0=biasc[:], in1=cnt[:], op=AX.add
                )
                # blt: bf16 [50, LT]: 0-23 bias_hi, 24-47 bias_lo, 48-49 = -1
                blt = wpool.tile([66, LT], bf16, tag="blt", bufs=2)
                nc.vector.memset(blt[:], -1.0)
                nc.vector.tensor_copy(blt[0:64, :], biasc[:])
                nc.vector.tensor_tensor(
                    out=blt[32:64, :], in0=biasc[32:64, :],
                    in1=blt[32:64, :], op=AX.subtract,
                )

                # hi/lo split of cnt (rows 0-23) for the mask rhs + chunk decay
                hi24 = wpool.tile([H, LT], bf16, tag="hi24", bufs=1)
                nc.any.tensor_copy(hi24[:], cnt[0:H, :])
                lo24 = wpool.tile([H, LT], bf16, tag="lo24", bufs=1)
                nc.vector.tensor_sub(lo24[:], cnt[0:H, :], hi24[:])
                dhi = dpool.tile([H, LT], bf16, tag="dhi")
                nc.sync.dma_start(out=dhi[:], in_=hi24[:])
                dlo = dpool.tile([H, LT], bf16, tag="dlo")
                nc.sync.dma_start(out=dlo[:], in_=lo24[:])

                wdtt = wpool.tile([H, LT], fp32, tag="wdtt", bufs=1)

                for c in range(NCH):
                    cs = slice(c * Q, (c + 1) * Q)
                    ci = li * NCH + c
                    lend = c * Q + Q - 1
                    rhs_p = rhs50
                    # hilo into mask rhs rows 48/49; chunk-end values for cdec
                    nc.sync.dma_start(
                        out=rhs_p[64:65, :].rearrange("p (h t) -> p h t", h=H),
                        in_=dhi[:, cs],
                    )
                    nc.sync.dma_start(
                        out=rhs_p[65:66, :].rearrange("p (h t) -> p h t", h=H),
                        in_=dlo[:, cs],
                    )
                    hilo_end = smpool.tile([2, H], bf16, tag="hiloe")
                    nc.sync.dma_start(
                        out=hilo_end[0:1, :], in_=dhi[:, lend : lend + 1]
                    )
                    nc.sync.dma_start(
                        out=hilo_end[1:2, :], in_=dlo[:, lend : lend + 1]
                    )

                    # w = exp(cnt - cnt_end) (decay to chunk end); wdt = dt*w
                    negend = smpool.tile([H, 1], fp32, tag="negend")
                    nc.vector.tensor_scalar(
                        out=negend[:], in0=cnt[0:H, lend : lend + 1],
                        scalar1=-1.0, scalar2=None, op0=AX.mult,
                    )
                    wv = smpool.tile([H, Q], fp32, tag="wv")
                    nc.scalar.activation(wv[:], cnt[0:H, cs], AF.Exp, bias=negend[:])
                    nc.vector.tensor_tensor(
                        out=wdtt[:, cs], in0=dtt[0:H, cs], in1=wv[:], op=AX.mult
                    )
                    # transpose wdt + cnt chunks -> token layout
                    pstw = psm.tile([128, H], fp32, tag="sm")
                    nc.tensor.transpose(pstw[:], wdtt[:, cs], ident_f[0:24, 0:24])
                    wdtT = smpool.tile([128, H], bf16, tag="wdtT")
                    nc.any.tensor_copy(wdtT[:], pstw[:])
                    pstc = psm.tile([128, H], fp32, tag="sm")
                    nc.tensor.transpose(pstc[:], cnt[0:H, cs], ident_f[0:24, 0:24])
                    expcum = smpool.tile([128, H], fp32, tag="expcum")
                    nc.scalar.activation(expcum[:], pstc[:], AF.Exp, scale=-1.0)
                    # chunk decay factors (64, 24)
                    pcd = psm.tile([64, H], fp32, tag="sm")
                    nc.tensor.matmul(
                        pcd[:], lhsT=pones2[:], rhs=hilo_end[:],
                        start=True, stop=True,
                    )
                    cdec = smpool.tile([64, H], fp32, tag="cdec")
                    nc.scalar.activation(cdec[:], pcd[:], AF.Exp, scale=-1.0)

                    B_ch = Bt[:, cs]
                    C_ch = Ct[:, cs]

                    # G^T (s,t) then causal mask; gtm4 = 4 dense copies
                    pgt = psm.tile([128, Q], fp32, tag="sm")
                    nc.tensor.matmul(pgt[:], lhsT=B_ch, rhs=C_ch, start=True, stop=True)
                    gtc = smpool.tile([128, Q], bf16, tag="gtc")
                    nc.any.tensor_copy(gtc[:], pgt[:])
                    gtm = smpool.tile([128, Q], bf16, tag="gtm")
                    nc.gpsimd.affine_select(
                        out=gtm[:], in_=gtc[:],
                        compare_op=AX.is_ge, fill=0.0,
                        base=0, pattern=[[1, Q]], channel_multiplier=-1,
                    )
                    gtm4 = smpool.tile([128, 4, Q], bf16, tag="gtm4", bufs=2)
                    nc.vector.tensor_copy(
                        gtm4[:], gtm[:].unsqueeze(1).broadcast_to([128, 4, Q])
                    )

                    # ---- mask: pbc = bias_hi_s + bias_lo_s - cnt_t per head ----
                    gmask24 = spool.tile([128, H * Q], bf16, tag="gmask24", bufs=1)
                    for r in range(3):
                        pmask = pmaskp.tile([128, 1024], fp32, tag="pm")
                        for q2 in range(2):
                            c0 = r * 1024 + q2 * 512
                            nc.tensor.matmul(
                                pmask[:, q2 * 512 : (q2 + 1) * 512],
                                lhsT=blt[:, cs],
                                rhs=rhs_p[:, c0 : c0 + 512],
                                start=True, stop=True,
                            )
                        mexr = smpool.tile([128, 1024], bf16, tag="mexr", bufs=1)
                        nc.scalar.activation(mexr[:], pmask[:], AF.Exp)
                        for q2 in range(2):
                            nc.vector.scalar_tensor_tensor(
                                out=gmask24[:, r * 1024 + q2 * 512 : r * 1024 + (q2 + 1) * 512],
                                in0=mexr[:, q2 * 512 : (q2 + 1) * 512],
                                scalar=1.0,
                                in1=gtm4[:],
                                op0=AX.min,
                                op1=AX.mult,
                            )

                    # B token layout
                    pbt = psm.tile([128, 64], bf16, tag="sm")
                    nc.tensor.transpose(pbt[:], B_ch, ident_b[0:64, 0:64])
                    btok = smpool.tile([128, 64], bf16, tag="btok")
                    nc.any.tensor_copy(btok[:], pbt[:])

                    # xs -> token layout (batched 4-transpose evacuation)
                    xst = spool.tile([128, D_INNER], bf16, tag="xst")
                    for g4 in range(3):
                        ptile = ptrp.tile([128, 4, 128], bf16, tag="tr")
                        for jj in range(4):
                            j = g4 * 4 + jj
                            nc.tensor.transpose(
                                ptile[:, jj, :], co_all[:, j, cs], ident_b[:]
                            )
                        nc.any.tensor_copy(
                            xst[:, g4 * 512 : (g4 + 1) * 512], ptile[:]
                        )

                    wdt_b = wdtT[:].unsqueeze(2).broadcast_to([128, H, 64])
                    wxd = spool.tile([128, H, 64], bf16, tag="wxd")
                    nc.vector.tensor_tensor(
                        out=wxd[:],
                        in0=xst[:].rearrange("p (h d) -> p h d", h=H),
                        in1=wdt_b,
                        op=AX.mult,
                    )

                    # ---- Y_inter (token layout) using S_prev ----
                    yw = spool.tile([128, D_INNER], bf16, tag="yw")
                    ecb = expcum[:].unsqueeze(2).broadcast_to([128, H, 64])
                    for j in range(NZ):
                        js = slice(j * 512, (j + 1) * 512)
                        pyi = pbig.tile([128, 512], fp32, tag="big")
                        nc.tensor.matmul(
                            pyi[:], lhsT=C_ch, rhs=S_b[:, js], start=True, stop=True
                        )
                        nc.vector.tensor_tensor(
                            out=yw[:, js].rearrange("p (h d) -> p h d", h=8),
                            in0=pyi[:].rearrange("p (h d) -> p h d", h=8),
                            in1=ecb[:, j * 8 : (j + 1) * 8, :],
                            op=AX.mult,
                        )

                    # ---- state update: S_new = cdec*S_old + Btok^T @ wXd ----
                    S_f_new = statepool.tile([64, H * 64], fp32, tag="Sf")
                    S_b_new = statepool.tile([64, H * 64], bf16, tag="Sb")
                    cdb = cdec[:].unsqueeze(2).broadcast_to([64, H, 64])
                    nc.vector.tensor_tensor(
                        out=S_f_new[:].rearrange("p (h d) -> p h d", h=H),
                        in0=S_f[:].rearrange("p (h d) -> p h d", h=H),
                        in1=cdb, op=AX.mult,
                    )
                    for g in range(3):
                        gs = slice(g * 512, (g + 1) * 512)
                        pds = pbig.tile([64, 512], fp32, tag="big")
                        nc.tensor.matmul(
                            pds[:], lhsT=btok[:],
                            rhs=wxd[:].rearrange("p h d -> p (h d)")[:, gs],
                            start=True, stop=True,
                        )
                        nc.vector.tensor_tensor(
                            out=S_f_new[:, gs], in0=S_f_new[:, gs], in1=pds[:],
                            op=AX.add,
                        )
                    nc.any.tensor_copy(S_b_new[:], S_f_new[:])
                    S_f, S_b = S_f_new, S_b_new

                    # ---- Y_intra ----
                    for j in range(NZ):
                        pyt = pbig.tile([128, 512], fp32, tag="big")
                        for h in range(j * 8, j * 8 + 8):
                            nc.tensor.matmul(
                                pyt[:, (h - j * 8) * 64 : (h - j * 8 + 1) * 64],
                                lhsT=gmask24[:, h * Q : (h + 1) * Q],
                                rhs=xst[:, h * 64 : (h + 1) * 64],
                                start=True, stop=True,
                            )
                        nc.vector.tensor_tensor(
                            out=yw[:, j * 512 : (j + 1) * 512],
                            in0=yw[:, j * 512 : (j + 1) * 512],
                            in1=pyt[:], op=AX.add,
                        )

                    # ---- skip D*xs, gate, RMS norm ----
                    dxs = spool.tile([128, D_INNER], bf16, tag="dxs")
                    nc.vector.tensor_tensor(out=dxs[:], in0=xst[:], in1=dsb_sb[:], op=AX.mult)
                    nc.vector.tensor_tensor(out=yw[:], in0=yw[:], in1=dxs[:], op=AX.add)
                    nc.vector.tensor_tensor(out=yw[:], in0=yw[:], in1=sz_lt[:, c, :], op=AX.mult)
                    sq = spool.tile([128, D_INNER], bf16, tag="dxs")
                    ssum = smpool.tile([128, 1], fp32, tag="ssum")
                    nc.scalar.activation(sq[:], yw[:], AF.Square, accum_out=ssum[:])
                    lnv = smpool.tile([128, 1], fp32, tag="lnv")
                    nc.scalar.activation(
                        lnv[:], ssum[:], AF.Ln, scale=1.0 / D_INNER, bias=eps_c[:]
                    )
                    rstd = smpool.tile([128, 1], fp32, tag="rstd")
                    nc.scalar.activation(rstd[:], lnv[:], AF.Exp, scale=-0.5)
                    yn = spool.tile([128, D_INNER], bf16, tag="xst")
                    nc.vector.tensor_scalar(
                        out=yn[:], in0=yw[:], scalar1=rstd[:], scalar2=None,
                        op0=AX.mult,
                    )

                    # ---- transpose yn (batched), final matmul ----
                    ynt = spool.tile([128, 12, 128], bf16, tag="wxd")
                    for g4 in range(3):
                        ptile = ptrp.tile([128, 4, 128], bf16, tag="tr")
                        for jj in range(4):
                            j = g4 * 4 + jj
                            nc.tensor.transpose(
                                ptile[:, jj, :], yn[:, j * 128 : (j + 1) * 128],
                                ident_b[:],
                            )
                        nc.any.tensor_copy(ynt[:, g4 * 4 : (g4 + 1) * 4, :], ptile[:])
                    osb = spool.tile([128, D_MODEL], fp32, tag="osb", bufs=1)
                    for n2 in range(2):
                        po = pbig.tile([128, 384], fp32, tag="big")
                        for j in range(12):
                            nc.tensor.matmul(
                                po[:],
                                lhsT=ynt[:, j, :],
                                rhs=wcb_sb[:, j, n2 * 384 : (n2 + 1) * 384],
                                start=(j == 0), stop=(j == 11),
                            )
                        nc.any.tensor_copy(osb[:, n2 * 384 : (n2 + 1) * 384], po[:])
                    nc.sync.dma_start(
                        out=out.ap()[l0 + c * Q : l0 + (c + 1) * Q, :], in_=osb[:]
                    )

    nc.finalize()
    return nc


def _prep_core_inputs(xb, p, flip):
    """Host-side preprocessing for one (branch, batch) core."""
    (in_w, conv_w, conv_b, dt_bias, A_log, Dp, norm_w, out_w, fus_half) = p
    x = xb[::-1] if flip else xb
    xT = np.ascontiguousarray(x.T).astype(ml_dtypes.bfloat16)
    wipT = np.ascontiguousarray(in_w.T)  # (768, 3224)
    z8 = np.zeros((D_MODEL, 8), wipT.dtype)
    dt24 = wipT[:, D_INNER + CONV_DIM :]
    wip_ext = np.concatenate([wipT, z8, dt24, z8], axis=1)
    wip = np.ascontiguousarray(wip_ext).astype(ml_dtypes.bfloat16)
    wcomb = (np.diag(norm_w.astype(np.float64)) @ out_w.T.astype(np.float64)
             @ fus_half.T.astype(np.float64)).astype(np.float32)
    wcb = wcomb.astype(ml_dtypes.bfloat16)
    cw = np.ascontiguousarray(conv_w[:, 0, :]).astype(np.float32)
    cb = conv_b.astype(np.float32)
    dtb = np.zeros((64, 1), np.float32)
    dtb[0:H, 0] = dt_bias
    dtb[32 : 32 + H, 0] = dt_bias
    apos = np.zeros((64, 1), np.float32)
    apos[0:H, 0] = np.exp(A_log)
    apos[32 : 32 + H, 0] = np.exp(A_log)
    dsb = np.broadcast_to(np.repeat(Dp, HEADDIM)[None, :], (128, D_INNER))
    dsb = np.ascontiguousarray(dsb).astype(ml_dtypes.bfloat16)
    # one-hot head selector rows (hi rows 0-23, lo rows 24-47)
    heads = np.arange(H * Q) // Q  # (3072,)
    ohm = np.zeros((64, H * Q), np.float32)
    for j in range(64):
        jj = j % 32
        if jj < H:
            ohm[j, heads == jj] = 1.0
    ohm = ohm.astype(ml_dtypes.bfloat16)
    return {
        "xT": xT, "wip": wip, "wcb": wcb, "cw": cw, "cb": cb,
        "dtb": dtb, "apos": apos, "dsb": dsb, "oh": ohm,
    }


def kernel(x, fus_w, fus_b,
           f_in_w, f_conv_w, f_conv_b, f_dt_bias, f_A_log, f_D, f_norm_w, f_out_w,
           b_in_w, b_conv_w, b_conv_b, b_dt_bias, b_A_log, b_D, b_norm_w, b_out_w):
    from concourse.bass_utils import run_bass_kernel_spmd

    if "nc" not in _CACHE:
        _CACHE["nc"] = _build_nc()
    nc = _CACHE["nc"]

    x = np.asarray(x, dtype=np.float32)
    fp = (f_in_w, f_conv_w, f_conv_b, f_dt_bias, f_A_log, f_D, f_norm_w, f_out_w,
          fus_w[:, :D_MODEL])
    bp = (b_in_w, b_conv_w, b_conv_b, b_dt_bias, b_A_log, b_D, b_norm_w, b_out_w,
          fus_w[:, D_MODEL:])
    fp = tuple(np.asarray(a) for a in fp)
    bp = tuple(np.asarray(a) for a in bp)

    in_maps = []
    for b in range(BATCH):
        in_maps.append(_prep_core_inputs(x[b], fp, flip=False))
    for b in range(BATCH):
        in_maps.append(_prep_core_inputs(x[b], bp, flip=True))

    res = run_bass_kernel_spmd(nc, in_maps, list(range(8)))
    out = np.empty((BATCH, SEQ, D_MODEL), np.float32)
    for b in range(BATCH):
        of = res.results[b]["out"]
        ob = res.results[BATCH + b]["out"][::-1]
        out[b] = of + ob + np.asarray(fus_b, np.float32)[None, :]
    return out


# revision 4
# speedup vs baseline: 1.3047x; 1.3047x over previous
"""BiMamba (fwd+bwd Mamba2 + fusion Linear) Trainium2 kernel — v2.

Sharding: 8 cores = 2 branches x 4 batches. Each core runs one full Mamba2
branch on one batch element via the chunked SSD formulation (chunk=128), with
the out-proj and fusion Linear folded into one matmul (W_comb). Host flips x
for the backward branch and sums the two per-branch partial outputs.

v2 changes vs v1:
- state update batched: 3 matmuls of N=512 instead of 24 per-head N=64
- mask exp bias folded into the broadcast matmul (48 bias rows hi/lo via a
  one-hot rhs block), one Exp per 8 heads instead of one per head
- conv: single batched SiLU per L-tile, batched halo handling, alignment-aware
  tap order
- batched PSUM evacuation of xs/yn transposes (4 transposes per copy)
- ywork in bf16; misc op batching
"""

import sys

sys.path.insert(0, "/opt/trn_rl_repo")

import numpy as np
import ml_dtypes

D_MODEL = 768
D_STATE = 64
D_CONV = 4
D_INNER = 1536
HEADDIM = 64
H = 24
CONV_DIM = D_INNER + 2 * D_STATE  # 1664
D_IN_PROJ = 2 * D_INNER + 2 * D_STATE + H  # 3224
D_IN_EXT = D_IN_PROJ + 40  # 3264: dt cols [3200:3264] = dt,pad8,dt,pad8
BATCH, SEQ = 4, 4096

LT = 512  # L-tile
NLT = SEQ // LT  # 8
Q = 128  # chunk
NCH = LT // Q  # chunks per L-tile
KT = D_MODEL // 128  # 6 k-tiles
MX = 13  # xBC m-tiles (1664/128)
NZ = D_INNER // 512  # 3 z slices
XROW = LT + 4  # padded xin row stride (keeps taps 4B-aligned)

_CACHE = {}


def _build_nc():
    import concourse.bass as bass
    import concourse.bacc as bacc
    import concourse.mybir as mybir
    from concourse.tile import TileContext
    from concourse.masks import make_identity

    fp32 = mybir.dt.float32
    bf16 = mybir.dt.bfloat16
    AX = mybir.AluOpType
    AF = mybir.ActivationFunctionType

    nc = bacc.Bacc("TRN2", debug=False, num_devices=8)

    xT = nc.declare_dram_parameter("xT", [D_MODEL, SEQ], bf16, isOutput=False)
    wip = nc.declare_dram_parameter("wip", [D_MODEL, D_IN_EXT], bf16, isOutput=False)
    wcb = nc.declare_dram_parameter("wcb", [D_INNER, D_MODEL], bf16, isOutput=False)
    cw = nc.declare_dram_parameter("cw", [CONV_DIM, D_CONV], fp32, isOutput=False)
    cb = nc.declare_dram_parameter("cb", [CONV_DIM], fp32, isOutput=False)
    dtb = nc.declare_dram_parameter("dtb", [64, 1], fp32, isOutput=False)
    apos = nc.declare_dram_parameter("apos", [64, 1], fp32, isOutput=False)
    dsb = nc.declare_dram_parameter("dsb", [128, D_INNER], bf16, isOutput=False)
    oh = nc.declare_dram_parameter("oh", [64, H * Q], bf16, isOutput=False)
    out = nc.declare_dram_parameter("out", [SEQ, D_MODEL], bf16, isOutput=True)

    with TileContext(nc) as tc:
        with (
            tc.tile_pool(name="const", bufs=1) as cpool,
            tc.tile_pool(name="xt", bufs=2) as xtpool,
            tc.tile_pool(name="work", bufs=2) as wpool,
            tc.tile_pool(name="conv", bufs=2) as convpool,
            tc.tile_pool(name="convin", bufs=1) as cinpool,
            tc.tile_pool(name="ssd", bufs=2) as spool,
            tc.tile_pool(name="ssdsm", bufs=4) as smpool,
            tc.tile_pool(name="state", bufs=2) as statepool,
            tc.tile_pool(name="dram", bufs=2, space="DRAM") as dpool,
            tc.tile_pool(name="pbig", bufs=3, space="PSUM") as pbig,
            tc.tile_pool(name="pmask", bufs=1, space="PSUM") as pmaskp,
            tc.tile_pool(name="ptr", bufs=2, space="PSUM") as ptrp,
            tc.tile_pool(name="psm", bufs=1, space="PSUM") as psm,
        ):
            # ---- constants ----
            wip_sb = cpool.tile([128, KT, D_IN_EXT], bf16, tag="wip")
            nc.sync.dma_start(
                out=wip_sb[:], in_=wip.ap().rearrange("(k p) m -> p k m", p=128)
            )
            wcb_sb = cpool.tile([128, 12, D_MODEL], bf16, tag="wcb")
            nc.sync.dma_start(
                out=wcb_sb[:], in_=wcb.ap().rearrange("(j p) m -> p j m", p=128)
            )
            cw_sb = cpool.tile([128, MX, D_CONV], fp32, tag="cw")
            nc.sync.dma_start(
                out=cw_sb[:], in_=cw.ap().rearrange("(a p) k -> p a k", p=128)
            )
            cb_sb = cpool.tile([128, MX], fp32, tag="cb")
            nc.sync.dma_start(
                out=cb_sb[:], in_=cb.ap().rearrange("(a p) -> p a", p=128)
            )
            dtb_sb = cpool.tile([64, 1], fp32, tag="dtb")
            nc.sync.dma_start(out=dtb_sb[:], in_=dtb.ap())
            apos_sb = cpool.tile([64, 1], fp32, tag="apos")
            nc.sync.dma_start(out=apos_sb[:], in_=apos.ap())
            dsb_sb = cpool.tile([128, D_INNER], bf16, tag="dsb")
            nc.sync.dma_start(out=dsb_sb[:], in_=dsb.ap())

            # persistent mask-matmul rhs tiles (double-buffered by chunk parity):
            # rows 0-47 one-hot head selector (constant), rows 48-49 hilo (per chunk)
            rhs50 = cpool.tile([66, H * Q], bf16, tag="rhs50")
            nc.sync.dma_start(out=rhs50[0:64, :], in_=oh.ap())

            ident_b = cpool.tile([128, 128], bf16, tag="idb")
            make_identity(nc, ident_b[:])
            ident_f = cpool.tile([32, 32], fp32, tag="idf")
            make_identity(nc, ident_f[:])
            ones66 = cpool.tile([66, 64], bf16, tag="ones66")
            nc.gpsimd.memset(ones66[:], 1.0)
            zer48 = cpool.tile([64, 128], bf16, tag="zer48")
            nc.gpsimd.memset(zer48[:], 0.0)
            eps_c = cpool.tile([128, 1], fp32, tag="eps")
            nc.gpsimd.memset(eps_c[:], 1e-5)
            one48 = cpool.tile([64, 1], fp32, tag="one48")
            nc.gpsimd.memset(one48[:], 1.0)

            # ---- loop-carried state ----
            S_f = statepool.tile([64, H * 64], fp32, tag="Sf")
            nc.vector.memset(S_f[:], 0.0)
            S_b = statepool.tile([64, H * 64], bf16, tag="Sb")
            nc.vector.memset(S_b[:], 0.0)

            halo = convpool.tile([128, MX, 3], bf16, tag="halo", name="halo")
            nc.vector.memset(halo[:], 0.0)

            for li in range(NLT):
                l0 = li * LT
                xtt = xtpool.tile([128, KT, LT], bf16, tag="xtt")
                nc.sync.dma_start(
                    out=xtt[:],
                    in_=xT.ap().rearrange("(k p) l -> p k l", p=128)[:, :, l0 : l0 + LT],
                )

                # ---- in_proj: xBC part (channel layout) ----
                xin = cinpool.tile([128, MX, XROW], bf16, tag="xin")
                nc.any.tensor_copy(xin[:, :, 0:3], halo[:])
                for m in range(MX):
                    ps = pbig.tile([128, LT], fp32, tag="big")
                    for k in range(KT):
                        nc.tensor.matmul(
                            ps[:],
                            lhsT=wip_sb[:, k, D_INNER + m * 128 : D_INNER + (m + 1) * 128],
                            rhs=xtt[:, k, :],
                            start=(k == 0),
                            stop=(k == KT - 1),
                        )
                    nc.any.tensor_copy(xin[:, m, 3 : LT + 3], ps[:])
                # new halo for next L-tile
                halo_new = convpool.tile([128, MX, 3], bf16, tag="halo")
                nc.any.tensor_copy(halo_new[:], xin[:, :, LT : LT + 3])
                halo = halo_new

                # ---- conv: 4 taps, tap3 via tensor_scalar (+bias), taps 2/1/0 STT
                co_all = convpool.tile([128, MX, LT], bf16, tag="co")
                for m in range(MX):
                    nc.vector.tensor_scalar(
                        out=co_all[:, m, :],
                        in0=xin[:, m, 3 : LT + 3],
                        scalar1=cw_sb[:, m, 3:4],
                        scalar2=cb_sb[:, m : m + 1],
                        op0=AX.mult,
                        op1=AX.add,
                    )
                    nc.vector.scalar_tensor_tensor(
                        out=co_all[:, m, :],
                        in0=xin[:, m, 1 : 1 + LT],
                        scalar=cw_sb[:, m, 1:2],
                        in1=co_all[:, m, :],
                        op0=AX.mult,
                        op1=AX.add,
                    )
                    for k in (2, 0):
                        nc.vector.scalar_tensor_tensor(
                            out=co_all[:, m, :],
                            in0=xin[:, m, k : k + LT],
                            scalar=cw_sb[:, m, k : k + 1],
                            in1=co_all[:, m, :],
                            op0=AX.mult,
                            op1=AX.add,
                        )
                nc.scalar.activation(co_all[:], co_all[:], AF.Silu)

                Bt = convpool.tile([64, LT], bf16, tag="Bt")
                nc.any.tensor_copy(Bt[:], co_all[0:64, 12, :])
                Ct = convpool.tile([64, LT], bf16, tag="Ct")
                nc.sync.dma_start(out=Ct[:], in_=co_all[64:128, 12, :])

                # ---- z part in_proj (token layout) + silu, whole L-tile ----
                sz_lt = wpool.tile([128, NCH, D_INNER], bf16, tag="szlt", bufs=1)
                for c4 in range(NCH):
                    c4s = slice(c4 * Q, (c4 + 1) * Q)
                    for j in range(NZ):
                        pz = pbig.tile([128, 512], fp32, tag="big")
                        for k in range(KT):
                            nc.tensor.matmul(
                                pz[:],
                                lhsT=xtt[:, k, c4s],
                                rhs=wip_sb[:, k, j * 512 : (j + 1) * 512],
                                start=(k == 0), stop=(k == KT - 1),
                            )
                        nc.scalar.activation(
                            sz_lt[:, c4, j * 512 : (j + 1) * 512], pz[:], AF.Silu
                        )

                osb4 = wpool.tile([128, NCH, D_MODEL], bf16, tag="osb4", bufs=1)

                # ---- in_proj: dt part (48 duplicated rows, channel layout) ----
                psdt = pbig.tile([64, LT], fp32, tag="big")
                for k in range(KT):
                    nc.tensor.matmul(
                        psdt[:],
                        lhsT=wip_sb[:, k, D_INNER + CONV_DIM : D_IN_EXT],
                        rhs=xtt[:, k, :],
                        start=(k == 0),
                        stop=(k == KT - 1),
                    )
                art = wpool.tile([64, LT], fp32, tag="art", bufs=1)
                nc.scalar.activation(art[:], psdt[:], AF.Exp, bias=dtb_sb[:])
                dtt = wpool.tile([64, LT], fp32, tag="dtt", bufs=1)
                nc.scalar.activation(dtt[:], art[:], AF.Ln, bias=one48[:])
                nc.vector.tensor_scalar(
                    out=art[:], in0=dtt[:], scalar1=apos_sb[:],
                    scalar2=None, op0=AX.mult,
                )
                cnt = wpool.tile([64, LT], fp32, tag="cnt", bufs=1)
                for c in range(NCH):
                    cs = slice(c * Q, (c + 1) * Q)
                    nc.vector.tensor_tensor_scan(
                        out=cnt[:, cs],
                        data0=art[:, cs],
                        data1=zer48[:],
                        initial=0.0,
                        op0=AX.add,
                        op1=AX.add,
                    )

                # biasc(rows 0-47) = ln(dt) + cnt (hi in 0-23, lo in 24-47 later)
                biasc = wpool.tile([64, LT], fp32, tag="art", bufs=1)
                nc.scalar.activation(biasc[:], dtt[:], AF.Ln)
                nc.vector.tensor_tensor(
                    out=biasc[:], in0=biasc[:], in1=cnt[:], op=AX.add
                )
                # blt: bf16 [50, LT]: 0-23 bias_hi, 24-47 bias_lo, 48-49 = -1
                blt = wpool.tile([66, LT], bf16, tag="blt", bufs=1)
                nc.vector.memset(blt[:], -1.0)
                nc.vector.tensor_copy(blt[0:64, :], biasc[:])
                nc.vector.tensor_tensor(
                    out=blt[32:64, :], in0=biasc[32:64, :],
                    in1=blt[32:64, :], op=AX.subtract,
                )

                # hi/lo split of cnt (rows 0-23) for the mask rhs + chunk decay
                hi24 = wpool.tile([H, LT], bf16, tag="hi24", bufs=1)
                nc.any.tensor_copy(hi24[:], cnt[0:H, :])
                lo24 = wpool.tile([H, LT], bf16, tag="lo24", bufs=1)
                nc.vector.tensor_sub(lo24[:], cnt[0:H, :], hi24[:])
                dhl = dpool.tile([2, H, LT], bf16, tag="dhl")
                nc.sync.dma_start(out=dhl[:, :, :][0:1], in_=hi24[:])
                nc.sync.dma_start(out=dhl[:, :, :][1:2], in_=lo24[:])

                wdtt = wpool.tile([H, LT], fp32, tag="wdtt", bufs=1)

                for c in range(NCH):
                    cs = slice(c * Q, (c + 1) * Q)
                    ci = li * NCH + c
                    lend = c * Q + Q - 1
                    rhs_p = rhs50
                    # hilo into mask rhs rows 64/65 (one DMA)
                    nc.sync.dma_start(
                        out=rhs_p[64:66, :].rearrange("p (h t) -> p h t", h=H),
                        in_=dhl[:, :, cs],
                    )

                    # w = exp(cnt - cnt_end) (decay to chunk end); wdt = dt*w
                    negend = smpool.tile([H, 1], fp32, tag="negend")
                    nc.vector.tensor_scalar(
                        out=negend[:], in0=cnt[0:H, lend : lend + 1],
                        scalar1=-1.0, scalar2=None, op0=AX.mult,
                    )
                    wv = smpool.tile([H, Q], fp32, tag="wv")
                    nc.scalar.activation(wv[:], cnt[0:H, cs], AF.Exp, bias=negend[:])
                    nc.vector.tensor_tensor(
                        out=wdtt[:, cs], in0=dtt[0:H, cs], in1=wv[:], op=AX.mult
                    )
                    # transpose wdt + cnt chunks -> token layout
                    pstw = psm.tile([128, H], fp32, tag="sm")
                    nc.tensor.transpose(pstw[:], wdtt[:, cs], ident_f[0:24, 0:24])
                    wdtT = smpool.tile([128, H], bf16, tag="wdtT")
                    nc.any.tensor_copy(wdtT[:], pstw[:])
                    pstc = psm.tile([128, H], fp32, tag="sm")
                    nc.tensor.transpose(pstc[:], cnt[0:H, cs], ident_f[0:24, 0:24])
                    expcum = smpool.tile([128, H], fp32, tag="expcum")
                    nc.scalar.activation(expcum[:], pstc[:], AF.Exp, scale=-1.0)
                    # chunk decay factors (64, 24)
                    pcd = psm.tile([64, H], fp32, tag="sm")
                    nc.tensor.matmul(
                        pcd[:], lhsT=ones66[64:66, :],
                        rhs=rhs_p[64:66, :].rearrange("p (h t) -> p h t", h=H)[
                            :, :, lend - c * Q
                        ],
                        start=True, stop=True,
                    )
                    cdec = smpool.tile([64, H], fp32, tag="cdec")
                    nc.scalar.activation(cdec[:], pcd[:], AF.Exp, scale=-1.0)

                    B_ch = Bt[:, cs]
                    C_ch = Ct[:, cs]

                    # G^T (s,t) then causal mask; gtm4 = 4 dense copies
                    pgt = psm.tile([128, Q], fp32, tag="sm")
                    nc.tensor.matmul(pgt[:], lhsT=B_ch, rhs=C_ch, start=True, stop=True)
                    gtc = smpool.tile([128, Q], bf16, tag="gtc")
                    nc.any.tensor_copy(gtc[:], pgt[:])
                    gtm = smpool.tile([128, Q], bf16, tag="gtm")
                    nc.gpsimd.affine_select(
                        out=gtm[:], in_=gtc[:],
                        compare_op=AX.is_ge, fill=0.0,
                        base=0, pattern=[[1, Q]], channel_multiplier=-1,
                    )
                    gtm4 = smpool.tile([128, 4, Q], bf16, tag="gtm4", bufs=1)
                    nc.vector.tensor_copy(
                        gtm4[:], gtm[:].unsqueeze(1).broadcast_to([128, 4, Q])
                    )

                    # ---- mask: pbc = bias_hi_s + bias_lo_s - cnt_t per head ----
                    gmask24 = spool.tile([128, H * Q], bf16, tag="gmask24", bufs=1)
                    for r in range(3):
                        pmask = pmaskp.tile([128, 1024], fp32, tag="pm")
                        for q2 in range(2):
                            c0 = r * 1024 + q2 * 512
                            nc.tensor.matmul(
                                pmask[:, q2 * 512 : (q2 + 1) * 512],
                                lhsT=blt[:, cs],
                                rhs=rhs_p[:, c0 : c0 + 512],
                                start=True, stop=True,
                            )
                        mexr = smpool.tile([128, 1024], bf16, tag="mexr", bufs=1)
                        nc.scalar.activation(mexr[:], pmask[:], AF.Exp)
                        for q2 in range(2):
                            nc.vector.scalar_tensor_tensor(
                                out=gmask24[:, r * 1024 + q2 * 512 : r * 1024 + (q2 + 1) * 512],
                                in0=mexr[:, q2 * 512 : (q2 + 1) * 512],
                                scalar=1.0,
                                in1=gtm4[:],
                                op0=AX.min,
                                op1=AX.mult,
                            )

                    # B token layout
                    pbt = psm.tile([128, 64], bf16, tag="sm")
                    nc.tensor.transpose(pbt[:], B_ch, ident_b[0:64, 0:64])
                    btok = smpool.tile([128, 64], bf16, tag="btok")
                    nc.any.tensor_copy(btok[:], pbt[:])

                    # xs -> token layout (batched 4-transpose evacuation)
                    xst = spool.tile([128, D_INNER], bf16, tag="xst")
                    for g4 in range(3):
                        ptile = ptrp.tile([128, 4, 128], bf16, tag="tr")
                        for jj in range(4):
                            j = g4 * 4 + jj
                            nc.tensor.transpose(
                                ptile[:, jj, :], co_all[:, j, cs], ident_b[:]
                            )
                        nc.any.tensor_copy(
                            xst[:, g4 * 512 : (g4 + 1) * 512], ptile[:]
                        )

                    wdt_b = wdtT[:].unsqueeze(2).broadcast_to([128, H, 64])
                    wxd = spool.tile([128, H, 64], bf16, tag="wxd")
                    nc.vector.tensor_tensor(
                        out=wxd[:],
                        in0=xst[:].rearrange("p (h d) -> p h d", h=H),
                        in1=wdt_b,
                        op=AX.mult,
                    )

                    # ---- Y_inter (token layout) using S_prev ----
                    yw = spool.tile([128, D_INNER], bf16, tag="yw")
                    ecb = expcum[:].unsqueeze(2).broadcast_to([128, H, 64])
                    for j in range(NZ):
                        js = slice(j * 512, (j + 1) * 512)
                        pyi = pbig.tile([128, 512], fp32, tag="big")
                        nc.tensor.matmul(
                            pyi[:], lhsT=C_ch, rhs=S_b[:, js], start=True, stop=True
                        )
                        nc.vector.tensor_tensor(
                            out=yw[:, js].rearrange("p (h d) -> p h d", h=8),
                            in0=pyi[:].rearrange("p (h d) -> p h d", h=8),
                            in1=ecb[:, j * 8 : (j + 1) * 8, :],
                            op=AX.mult,
                        )

                    # ---- state update: S_new = cdec*S_old + Btok^T @ wXd ----
                    S_f_new = statepool.tile([64, H * 64], fp32, tag="Sf")
                    S_b_new = statepool.tile([64, H * 64], bf16, tag="Sb")
                    cdb = cdec[:].unsqueeze(2).broadcast_to([64, H, 64])
                    nc.vector.tensor_tensor(
                        out=S_f_new[:].rearrange("p (h d) -> p h d", h=H),
                        in0=S_f[:].rearrange("p (h d) -> p h d", h=H),
                        in1=cdb, op=AX.mult,
                    )
                    for g in range(3):
                        gs = slice(g * 512, (g + 1) * 512)
                        pds = pbig.tile([64, 512], fp32, tag="big")
                        nc.tensor.matmul(
                            pds[:], lhsT=btok[:],
                            rhs=wxd[:].rearrange("p h d -> p (h d)")[:, gs],
                            start=True, stop=True,
                        )
                        nc.vector.tensor_tensor(
                            out=S_f_new[:, gs], in0=S_f_new[:, gs], in1=pds[:],
                            op=AX.add,
                        )
                    nc.any.tensor_copy(S_b_new[:], S_f_new[:])
                    S_f, S_b = S_f_new, S_b_new

                    # ---- Y_intra ----
                    for j in range(NZ):
                        pyt = pbig.tile([128, 512], fp32, tag="big")
                        for h in range(j * 8, j * 8 + 8):
                            nc.tensor.matmul(
                                pyt[:, (h - j * 8) * 64 : (h - j * 8 + 1) * 64],
                                lhsT=gmask24[:, h * Q : (h + 1) * Q],
                                rhs=xst[:, h * 64 : (h + 1) * 64],
                                start=True, stop=True,
                            )
                        nc.vector.tensor_tensor(
                            out=yw[:, j * 512 : (j + 1) * 512],
                            in0=yw[:, j * 512 : (j + 1) * 512],
                            in1=pyt[:], op=AX.add,
                        )

                    # ---- skip D*xs, gate, RMS norm ----
                    dxs = spool.tile([128, D_INNER], bf16, tag="dxs")
                    nc.vector.tensor_tensor(out=dxs[:], in0=xst[:], in1=dsb_sb[:], op=AX.mult)
                    nc.vector.tensor_tensor(out=yw[:], in0=yw[:], in1=dxs[:], op=AX.add)
                    nc.vector.tensor_tensor(out=yw[:], in0=yw[:], in1=sz_lt[:, c, :], op=AX.mult)
                    sq = spool.tile([128, D_INNER], bf16, tag="dxs")
                    ssum = smpool.tile([128, 1], fp32, tag="ssum")
                    nc.scalar.activation(sq[:], yw[:], AF.Square, accum_out=ssum[:])
                    lnv = smpool.tile([128, 1], fp32, tag="lnv")
                    nc.scalar.activation(
                        lnv[:], ssum[:], AF.Ln, scale=1.0 / D_INNER, bias=eps_c[:]
                    )
                    rstd = smpool.tile([128, 1], fp32, tag="rstd")
                    nc.scalar.activation(rstd[:], lnv[:], AF.Exp, scale=-0.5)
                    yn = spool.tile([128, D_INNER], bf16, tag="xst")
                    nc.vector.tensor_scalar(
                        out=yn[:], in0=yw[:], scalar1=rstd[:], scalar2=None,
                        op0=AX.mult,
                    )

                    # ---- transpose yn (batched), final matmul ----
                    ynt = spool.tile([128, 12, 128], bf16, tag="wxd")
                    for g4 in range(3):
                        ptile = ptrp.tile([128, 4, 128], bf16, tag="tr")
                        for jj in range(4):
                            j = g4 * 4 + jj
                            nc.tensor.transpose(
                                ptile[:, jj, :], yn[:, j * 128 : (j + 1) * 128],
                                ident_b[:],
                            )
                        nc.any.tensor_copy(ynt[:, g4 * 4 : (g4 + 1) * 4, :], ptile[:])
                    for n2 in range(2):
                        po = pbig.tile([128, 384], fp32, tag="big")
                        for j in range(12):
                            nc.tensor.matmul(
                                po[:],
                                lhsT=ynt[:, j, :],
                                rhs=wcb_sb[:, j, n2 * 384 : (n2 + 1) * 384],
                                start=(j == 0), stop=(j == 11),
                            )
                        nc.any.tensor_copy(
                            osb4[:, c, n2 * 384 : (n2 + 1) * 384], po[:]
                        )
                if True:
                    nc.sync.dma_start(
                        out=out.ap()[l0 : l0 + LT, :].rearrange(
                            "(c p) m -> p c m", p=128
                        ),
                        in_=osb4[:],
                    )

    nc.finalize()
    return nc


def _prep_core_inputs(xb, p, flip):
    """Host-side preprocessing for one (branch, batch) core."""
    (in_w, conv_w, conv_b, dt_bias, A_log, Dp, norm_w, out_w, fus_half) = p
    x = xb[::-1] if flip else xb
    xT = np.ascontiguousarray(x.T).astype(ml_dtypes.bfloat16)
    wipT = np.ascontiguousarray(in_w.T)  # (768, 3224)
    z8 = np.zeros((D_MODEL, 8), wipT.dtype)
    dt24 = wipT[:, D_INNER + CONV_DIM :]
    wip_ext = np.concatenate([wipT, z8, dt24, z8], axis=1)
    wip = np.ascontiguousarray(wip_ext).astype(ml_dtypes.bfloat16)
    wcomb = (np.diag(norm_w.astype(np.float64)) @ out_w.T.astype(np.float64)
             @ fus_half.T.astype(np.float64)).astype(np.float32)
    wcb = wcomb.astype(ml_dtypes.bfloat16)
    cw = np.ascontiguousarray(conv_w[:, 0, :]).astype(np.float32)
    cb = conv_b.astype(np.float32)
    dtb = np.zeros((64, 1), np.float32)
    dtb[0:H, 0] = dt_bias
    dtb[32 : 32 + H, 0] = dt_bias
    apos = np.zeros((64, 1), np.float32)
    apos[0:H, 0] = np.exp(A_log)
    apos[32 : 32 + H, 0] = np.exp(A_log)
    dsb = np.broadcast_to(np.repeat(Dp, HEADDIM)[None, :], (128, D_INNER))
    dsb = np.ascontiguousarray(dsb).astype(ml_dtypes.bfloat16)
    # one-hot head selector rows (hi rows 0-23, lo rows 24-47)
    heads = np.arange(H * Q) // Q  # (3072,)
    ohm = np.zeros((64, H * Q), np.float32)
    for j in range(64):
        jj = j % 32
        if jj < H:
            ohm[j, heads == jj] = 1.0
    ohm = ohm.astype(ml_dtypes.bfloat16)
    return {
        "xT": xT, "wip": wip, "wcb": wcb, "cw": cw, "cb": cb,
        "dtb": dtb, "apos": apos, "dsb": dsb, "oh": ohm,
    }


def kernel(x, fus_w, fus_b,
           f_in_w, f_conv_w, f_conv_b, f_dt_bias, f_A_log, f_D, f_norm_w, f_out_w,
           b_in_w, b_conv_w, b_conv_b, b_dt_bias, b_A_log, b_D, b_norm_w, b_out_w):
    from concourse.bass_utils import run_bass_kernel_spmd

    if "nc" not in _CACHE:
        _CACHE["nc"] = _build_nc()
    nc = _CACHE["nc"]

    x = np.asarray(x, dtype=np.float32)
    fp = (f_in_w, f_conv_w, f_conv_b, f_dt_bias, f_A_log, f_D, f_norm_w, f_out_w,
          fus_w[:, :D_MODEL])
    bp = (b_in_w, b_conv_w, b_conv_b, b_dt_bias, b_A_log, b_D, b_norm_w, b_out_w,
          fus_w[:, D_MODEL:])
    fp = tuple(np.asarray(a) for a in fp)
    bp = tuple(np.asarray(a) for a in bp)

    in_maps = []
    for b in range(BATCH):
        in_maps.append(_prep_core_inputs(x[b], fp, flip=False))
    for b in range(BATCH):
        in_maps.append(_prep_core_inputs(x[b], bp, flip=True))

    res = run_bass_kernel_spmd(nc, in_maps, list(range(8)))
    out = np.empty((BATCH, SEQ, D_MODEL), np.float32)
    for b in range(BATCH):
        of = np.asarray(res.results[b]["out"], np.float32)
        ob = np.asarray(res.results[BATCH + b]["out"], np.float32)[::-1]
        out[b] = of + ob + np.asarray(fus_b, np.float32)[None, :]
    return out
